# revision 26
# baseline (speedup 1.0000x reference)
"""Trainium2 Bass kernel for an image-captioning LSTM decoder.

Model (per reference):
  emb = embedding[captions]                      [B, T, E]
  sum_enc = encoder_out.sum(axis=1)              [B, ENC]
  h0 = mean_enc @ W_h0.T + b_h0 ; c0 likewise
  per step t (Tdec = T-1 steps):
    gates = [emb_t, sum_enc] @ W_ih.T + b_ih + h @ W_hh.T + b_hh
    i,f,g,o -> LSTM update; rows with t >= caption_len-1 give preds 0
    preds_t = h_new @ W_fc.T + b_fc  (masked)

Sharding: data-parallel over batch: core c owns rows {c, c+8, ...}.

Per-core design (v2):
  - EG[t] = emb_t@W_ih[:, :E].T + sum_enc@W_ih[:, E:].T + biases precomputed
    for all steps as one GEMM (phase B), kept RESIDENT in SBUF (f32r),
    compact layout [128 positions(16 steps), 2048].
  - Gates PSUM is quarter-packed [128, 512]: gate q-map [g@0, o@32, i@64,
    f@96], each quarter holds 4 replicas of the 8 batch rows.  EG is folded
    into PSUM by a selector matmul (SEL lhsT, tile_position), then 4 k-chunk
    matmuls accumulate W_hh·h (HTrep stationary [128,32]).
  - Pointwise tail reads PSUM quarters directly (mixed-space ops dodge the
    equal-base rule): tanh g -> Bt[0:32]; sig [i|f] in one [64] ACT op;
    t1 = i*g (DVE), t2 = f*c (Pool) in parallel; c' = t1+t2; tanh; h =
    stt(tanh_c, mask, sig_o).  All ops are d-halved (256) so the next
    step's k0/k1 gate matmuls start as soon as HTrep k0/k1 are cast.
  - W_fc resident in SBUF as bf16 (80KB/partition); h additionally cast to
    bf16 (HTb) for the fc GEMM.  fc chunks (500 vocab cols) interleave into
    PE gaps; outputs stream to HBM per chunk.
"""

import numpy as np
from contextlib import ExitStack

import concourse.bass as bass
import concourse.tile as tile
from concourse import mybir, bacc
from concourse.bass_utils import run_bass_kernel_spmd
from concourse.masks import make_identity

F32 = mybir.dt.float32
F32R = mybir.dt.float32r
BF16 = mybir.dt.bfloat16
SIG = mybir.ActivationFunctionType.Sigmoid
TANH = mybir.ActivationFunctionType.Tanh

NCORES = 8
B, T, V, E, D, ENC, P = 64, 64, 10000, 512, 512, 512, 196
TD = T - 1            # 63 decode steps
R = B // NCORES       # 8 rows per core
POS = TD * R          # 504 positions per core
KD = D // 128         # 4 k-tiles of the hidden dim
VCH = 500             # vocab chunk (psum bank holds 512 fp32)
NV = V // VCH         # 20 chunks
MTS = [128, 128, 128, POS - 384]   # fc position tiles (504 = 3*128 + 120)

_PROG_CACHE = {}


def _build_program(with_bfc: bool):
    nc = bacc.Bacc("TRN2", target_bir_lowering=False, debug=False,
                   num_devices=NCORES)

    def inp(name, shape, dt=F32):
        return nc.dram_tensor(name, shape, dt, kind="ExternalInput").ap()

    enc = inp("enc", [R, P, ENC], F32R)
    embT = inp("embT", [KD, 128, 512], F32R)
    wembT = inp("wembT", [KD, 128, 4 * D], F32R)
    wencT = inp("wencT", [KD, 128, 4 * D], F32R)
    whhT = inp("whhT", [KD, 128, 4 * D], F32R)
    wh0T = inp("wh0T", [KD, 128, D], F32R)
    wc0T = inp("wc0T", [KD, 128, D], F32R)
    wfcT = inp("wfcT", [KD, 128, V], BF16)
    bsum8 = inp("bsum8", [R, 4 * D])
    bh032 = inp("bh032", [32, D])
    bc032 = inp("bc032", [32, D])
    mask32 = inp("mask32", [32, TD])
    rowsel = inp("rowsel", [128, R * R], F32R)
    selm = inp("selm", [128, 16 * 32], F32R)
    irep8 = inp("irep8", [8, 32], F32R)
    if with_bfc:
        bfc = inp("bfc", [1, V], BF16)
        mask1 = inp("mask1", [1, POS], BF16)
    preds = nc.dram_tensor("preds", [R, TD, V], F32, kind="ExternalOutput").ap()
    preds_trv = preds.rearrange("r t v -> t r v")

    with tile.TileContext(nc) as tc, ExitStack() as ctx:
        const_pool = ctx.enter_context(tc.tile_pool(name="const", bufs=1))
        state_pool = ctx.enter_context(tc.tile_pool(name="state", bufs=1))

        SEL = const_pool.tile([128, 16 * 32], F32R, name="SEL")
        nc.sync.dma_start(SEL[:], selm[:])
        IREP = const_pool.tile([8, 32], F32R, name="IREP")
        nc.sync.dma_start(IREP[:], irep8[:])
        maskb = const_pool.tile([32, TD], F32, name="maskb")
        nc.sync.dma_start(maskb[:], mask32[:])

        ENGS = [nc.sync, nc.gpsimd, nc.scalar, nc.sync]
        whh_sb = [const_pool.tile([128, 4 * D], F32R, name=f"whh{k}")
                  for k in range(KD)]
        for k in range(KD):
            ENGS[k].dma_start(whh_sb[k][:], whhT[k])

        # persistent state
        EG = [state_pool.tile([128, 4 * D], F32R, name=f"EG{m}")
              for m in range(4)]

        HTb = [state_pool.tile([128, R * T], BF16, name=f"HTb{k}")
               for k in range(KD)]
        Gt = state_pool.tile([32, D], F32, name="Gt")   # tanh(g)
        Ct = state_pool.tile([32, D], F32, name="Ct")   # c state
        xenc16 = state_pool.tile([128, 4 * D], F32, name="xenc16")
        h0_sb = state_pool.tile([32, D], F32R, name="h0_sb")
        HTrep = [state_pool.tile([128, 128], F32R, name=f"HTrep{s}")
                 for s in range(2)]

        # ---- Phase A: encoder row sums, h0/c0, xenc ----
        with tc.tile_pool(name="pha", bufs=1) as pha, \
             tc.tile_pool(name="phas", bufs=2) as phas, \
             tc.tile_pool(name="pha_ps", bufs=1, space="PSUM") as pha_ps, \
             tc.tile_pool(name="phb_ps", bufs=1, space="PSUM") as phb_ps:
            rsel = pha.tile([128, R * R], F32R, name="rsel")
            nc.sync.dma_start(rsel[:], rowsel[:])

            sum_ps = pha_ps.tile([R, ENC], F32, name="sum_ps")
            nmm = 0
            for r in range(R):
                for c2, (p0, pn) in enumerate(((0, 128), (128, P - 128))):
                    et = phas.tile([128, ENC], F32R, name=f"enc{r}_{c2}",
                                  tag="encchunk")
                    ENGS[r % 4].dma_start(et[:pn, :], enc[r, p0:p0 + pn, :])
                    nc.tensor.matmul(sum_ps[:], rsel[:pn, 8 * r:8 * r + R],
                                     et[:pn, :], start=(nmm == 0),
                                     stop=(nmm == 2 * R - 1))
                    nmm += 1
            sum_sb = pha.tile([R, ENC], F32R, name="sum_sb")
            nc.vector.tensor_copy(sum_sb[:], sum_ps[:])

            # sumTrep[k]: [128, 32] transposed sums, 4 replicas
            sumTrep = []
            for k in range(KD):
                tp = phb_ps.tile([128, 32], F32, name=f"sumT_ps{k}", tag="tps")
                nc.tensor.matmul(tp[:], sum_sb[:, 128 * k:128 * (k + 1)],
                                 IREP[:], start=True, stop=True)
                st = pha.tile([128, 32], F32R, name=f"sumTrep{k}")
                nc.vector.tensor_copy(st[:], tp[:])
                sumTrep.append(st)

            # h0 / c0 (weights pre-scaled by 1/196 on host), [32, D] replicated
            for name, wT, b32 in (("h0", wh0T, bh032), ("c0", wc0T, bc032)):
                ps = phb_ps.tile([32, D], F32, name=f"{name}_ps", tag="h0ps")
                for k in range(KD):
                    wt = phas.tile([128, D], F32R, name=f"w{name}{k}",
                                   tag="w0chunk")
                    ENGS[k].dma_start(wt[:], wT[k])
                    nc.tensor.matmul(ps[:], sumTrep[k][:], wt[:],
                                     start=(k == 0), stop=(k == KD - 1))
                bt = pha.tile([32, D], F32, name=f"b{name}", tag="b0chunk")
                nc.sync.dma_start(bt[:], b32[:])
                if name == "h0":
                    nc.vector.tensor_add(h0_sb[:], ps[:], bt[:])
                else:
                    nc.vector.tensor_add(Ct[:], ps[:], bt[:])

            # xenc = sum_enc @ W_ih[:, E:].T + bsum  -> [8, 2048]
            xe_ps = pha_ps.tile([R, 4 * D], F32, name="xe_ps", tag="bigps")
            for k in range(KD):
                wt = phas.tile([128, 4 * D], F32R, name=f"wenc{k}",
                               tag="wencchunk")
                ENGS[k].dma_start(wt[:], wencT[k])
                for n in range(4):
                    nc.tensor.matmul(xe_ps[:, D * n:D * (n + 1)],
                                     sumTrep[k][:, 0:8],
                                     wt[:, D * n:D * (n + 1)],
                                     start=(k == 0), stop=(k == KD - 1))
            bs = pha.tile([R, 4 * D], F32, name="bsum_sb")
            nc.sync.dma_start(bs[:], bsum8[:])
            xenc = pha.tile([R, 4 * D], F32, name="xenc")
            nc.vector.tensor_add(xenc[:], xe_ps[:], bs[:])
            nc.sync.dma_start(xenc16[0:8, :], xenc[:])
            for w in (8, 16, 32, 64):
                nc.sync.dma_start(xenc16[w:2 * w, :], xenc16[0:w, :])

            # h0 -> HTrep[0] via replicated transpose (do inside step pools)

        # ---- Phase B: EG = embT.T @ wembT (+xenc16) -> SBUF resident ----
        with tc.tile_pool(name="phb", bufs=1) as phb, \
             tc.tile_pool(name="phb2_ps", bufs=2, space="PSUM") as phb2_ps:
            emb_sb = [phb.tile([128, 512], F32R, name=f"emb_sb{k}")
                      for k in range(KD)]
            wemb_sb = [phb.tile([128, 4 * D], F32R, name=f"wemb_sb{k}")
                       for k in range(KD)]
            for k in range(KD):
                ENGS[k].dma_start(emb_sb[k][:], embT[k])
                ENGS[(k + 2) % 4].dma_start(wemb_sb[k][:], wembT[k])
            for m in range(4):
                eg_ps = phb2_ps.tile([128, 4 * D], F32, name=f"eg_ps{m}",
                                     tag="egps")
                for n in range(4):
                    for k in range(KD):
                        nc.tensor.matmul(
                            eg_ps[:, D * n:D * (n + 1)],
                            emb_sb[k][:, 128 * m:128 * (m + 1)],
                            wemb_sb[k][:, D * n:D * (n + 1)],
                            start=(k == 0), stop=(k == KD - 1))
                nc.vector.tensor_add(EG[m][:, :], eg_ps[:, :],
                                     xenc16[:, :])

        # ---- Steps ----
        with tc.tile_pool(name="wfc", bufs=1) as wfc_pool, \
             tc.tile_pool(name="phc", bufs=2) as phc, \
             tc.tile_pool(name="phd_out", bufs=3) as phd_out, \
             tc.tile_pool(name="gps", bufs=1, space="PSUM") as gps_pool, \
             tc.tile_pool(name="tps", bufs=1, space="PSUM") as tps_pool, \
             tc.tile_pool(name="fps", bufs=3, space="PSUM") as fps_pool:

            wfc_sb = [wfc_pool.tile([128, V], BF16, name=f"wfc{k}")
                      for k in range(KD)]
            ENGS2 = [nc.sync, nc.gpsimd, nc.scalar, nc.gpsimd]
            for k in range(KD):
                ENGS2[k].dma_start(wfc_sb[k][:], wfcT[k])
            if with_bfc:
                bfc_sb = wfc_pool.tile([1, V], BF16, name="bfc_sb")
                nc.sync.dma_start(bfc_sb[:], bfc[:])
                m1_sb = wfc_pool.tile([1, POS], BF16, name="m1_sb")
                nc.sync.dma_start(m1_sb[:], mask1[:])

            # h0 -> HTrep[0] via replicated transposes
            init_tp = tps_pool.tile([128, 128], F32, name="init_tp", tag="tp")
            for k in range(KD):
                nc.tensor.matmul(init_tp[:, 32 * k:32 * (k + 1)],
                                 h0_sb[0:8, 128 * k:128 * (k + 1)], IREP[:],
                                 start=True, stop=True)
            nc.vector.tensor_copy(HTrep[0][:], init_tp[:])

            def fc_mm(m, n):
                """fc GEMM for preds[postile m, vocab chunk n] -> psum."""
                mw = MTS[m]
                ps = fps_pool.tile([128, VCH], F32, name=f"fc{n}_{m}",
                                   tag="fcps")
                for k in range(KD):
                    nc.tensor.matmul(
                        ps[:mw, :],
                        HTb[k][:, R + 128 * m:R + 128 * m + mw],
                        wfc_sb[k][:, VCH * n:VCH * (n + 1)], start=(k == 0),
                        stop=(k == KD - 1 and not with_bfc))
                if with_bfc:
                    nc.tensor.matmul(
                        ps[:mw, :], m1_sb[:, 128 * m:128 * m + mw],
                        bfc_sb[:, VCH * n:VCH * (n + 1)],
                        start=False, stop=True)
                return ps

            def fc_out(ps, m, n):
                mw = MTS[m]
                ot = phd_out.tile([128, VCH], F32, name=f"fo{n}_{m}",
                                  tag="fcout")
                nc.vector.tensor_copy(ot[:mw, :], ps[:mw, :])
                t0 = 16 * m
                tn = mw // R
                nc.sync.dma_start(
                    preds_trv[t0:t0 + tn, :, VCH * n:VCH * (n + 1)],
                    ot[:mw, :])

            def emit_fc_mm(t):
                out = []
                m = t // 16 - 1
                if m >= 0:
                    L = min(16 * (m + 2), TD) - 16 * (m + 1)
                    s = t - 16 * (m + 1)
                    for n in range(s * NV // L, (s + 1) * NV // L):
                        out.append((fc_mm(m, n), m, n))
                return out

            GORD = (0, 3, 2, 1)  # emission order: g, f, i, o


            def bankof(t):
                return [gps_pool.tile([32, D], F32, name=f"ps{t}_{g}",
                                      tag=f"gps{g}")
                        for g in range(4)]

            banks = {}

            def folds(t):
                j = t % 16
                mt = t // 16
                banks[t] = bankof(t)
                for fsl in (slice(0, 256), slice(256, 512)):
                    for g in GORD:
                        gsl = slice(512 * g + fsl.start,
                                    512 * g + fsl.stop)
                        nc.tensor.matmul(banks[t][g][:, fsl],
                                         SEL[:, 32 * j:32 * (j + 1)],
                                         EG[mt][:, gsl],
                                         start=True, stop=False)

            def gates(t, ks):
                htr = HTrep[t % 2]
                for fsl in (slice(0, 256), slice(256, 512)):
                    for g in GORD:
                        gsl = slice(512 * g + fsl.start,
                                    512 * g + fsl.stop)
                        for k in ks:
                            nc.tensor.matmul(banks[t][g][:, fsl],
                                             htr[k][:], whh_sb[k][:, gsl],
                                             start=False, stop=(k == KD - 1))

            hprev = [None]
            for t in range(TD):
                htr = HTrep[t % 2]
                htn = HTrep[(t + 1) % 2]
                j = t % 16
                mt = t // 16
                banks[t] = bankof(t)
                bank0 = banks[t]
                # --- PE stream: fill, transposes(t-1)+casts, fc, gates ---
                if t > 0:
                    hp, tpp = hprev[0]
                    for k in range(KD):
                        nc.tensor.matmul(tpp[:, 32 * k:32 * (k + 1)],
                                         hp[0:8, 128 * k:128 * (k + 1)],
                                         IREP[:], start=True, stop=True)
                    nc.vector.tensor_copy(htr[:], tpp[:])
                    for k in range(KD):
                        nc.gpsimd.tensor_copy(
                            HTb[k][:, R * t:R * (t + 1)],
                            htr[:, 32 * k:32 * k + 8])
                fcq = emit_fc_mm(t)
                for fsl in (slice(0, 256), slice(256, 512)):
                    for g in GORD:
                        gsl = slice(512 * g + fsl.start, 512 * g + fsl.stop)
                        nc.tensor.matmul(bank0[g][:, fsl],
                                         SEL[:, 32 * j:32 * (j + 1)],
                                         EG[mt][:, gsl],
                                         start=True, stop=False)
                        for k in range(KD):
                            nc.tensor.matmul(bank0[g][:, fsl],
                                             htr[:, 32 * k:32 * (k + 1)],
                                             whh_sb[k][:, gsl],
                                             start=False, stop=(k == KD - 1))
                bank = banks[t]
                a_i = phc.tile([32, D], F32, name=f"ai{t}", tag="ai")
                a_f = phc.tile([32, D], F32, name=f"af{t}", tag="af")
                acto = phc.tile([32, D], F32, name=f"ao{t}", tag="ao")
                t1s = phc.tile([32, D], F32, name=f"t1_{t}", tag="t1s")
                t2s = phc.tile([32, D], F32, name=f"t2_{t}", tag="t2s")
                tc_t = phc.tile([32, D], F32, name=f"tc{t}", tag="tct")
                h_sb = phc.tile([32, D], F32R, name=f"h{t}", tag="hsb")
                tp = tps_pool.tile([128, 128], F32, name=f"tp{t}", tag="tp")
                F0, F1 = slice(0, 256), slice(256, 512)
                for h2, fsl in ((0, F0), (1, F1)):
                    nc.scalar.activation(Gt[:, fsl], bank[0][:, fsl], TANH)
                    nc.scalar.activation(a_f[:, fsl], bank[3][:, fsl], SIG)
                    nc.scalar.activation(a_i[:, fsl], bank[2][:, fsl], SIG)
                    if h2 == 0:
                        nc.scalar.activation(acto[:, fsl], bank[1][:, fsl],
                                             SIG)
                    nc.gpsimd.tensor_mul(t2s[:, fsl], a_f[:, fsl],
                                         Ct[:, fsl])
                    nc.vector.tensor_mul(t1s[:, fsl], a_i[:, fsl],
                                         Gt[:, fsl])
                    nc.vector.tensor_add(Ct[:, fsl], t1s[:, fsl],
                                         t2s[:, fsl])
                nc.scalar.activation(tc_t[:, F0], Ct[:, F0], TANH)
                nc.scalar.activation(acto[:, F1], bank[1][:, F1], SIG)
                nc.scalar.activation(tc_t[:, F1], Ct[:, F1], TANH)
                for h2, fsl in ((0, F0), (1, F1)):
                    nc.vector.scalar_tensor_tensor(
                        h_sb[:, fsl], tc_t[:, fsl], maskb[:, t:t + 1],
                        acto[:, fsl], mybir.AluOpType.mult,
                        mybir.AluOpType.mult)
                hprev[0] = (h_sb, tp)
                for args in fcq:
                    fc_out(*args)

            # last step's h -> HTb col TD (for the final fc m=3 chunks)
            hp, tpp = hprev[0]
            for k in range(KD):
                nc.tensor.matmul(tpp[:, 32 * k:32 * (k + 1)],
                                 hp[0:8, 128 * k:128 * (k + 1)],
                                 IREP[:], start=True, stop=True)
            nc.vector.tensor_copy(HTrep[TD % 2][:], tpp[:])
            for k in range(KD):
                nc.gpsimd.tensor_copy(HTb[k][:, R * TD:R * (TD + 1)],
                                      HTrep[TD % 2][:, 32 * k:32 * k + 8])

            for n in range(NV):
                fc_out(fc_mm(3, n), 3, n)

    nc.compile()
    return nc


GPERM = None  # row permutation [g, o, i, f] built lazily


def _gate_perm():
    global GPERM
    if GPERM is None:
        GPERM = np.concatenate([np.arange(2 * D, 3 * D),
                                np.arange(3 * D, 4 * D),
                                np.arange(0, D), np.arange(D, 2 * D)])
    return GPERM


def _chunkT(w):
    """[N, K<=512] weight -> transposed chunks [KD, 128, N] (contiguous)."""
    wt = np.ascontiguousarray(w.T.astype(np.float32))
    return wt.reshape(KD, 128, w.shape[0])


def _bf16(a):
    import ml_dtypes
    return a.astype(ml_dtypes.bfloat16)


def kernel(encoder_out, encoder_captions, caption_len, embedding,
           W_ih, b_ih, W_hh, b_hh, W_h0, b_h0, W_c0, b_c0, W_fc, b_fc):
    encoder_out = np.asarray(encoder_out, dtype=np.float32)
    encoder_captions = np.asarray(encoder_captions)
    caption_len = np.asarray(caption_len)
    embedding = np.asarray(embedding, dtype=np.float32)
    W_ih = np.asarray(W_ih, dtype=np.float32); b_ih = np.asarray(b_ih, np.float32)
    W_hh = np.asarray(W_hh, dtype=np.float32); b_hh = np.asarray(b_hh, np.float32)
    W_h0 = np.asarray(W_h0, dtype=np.float32); b_h0 = np.asarray(b_h0, np.float32)
    W_c0 = np.asarray(W_c0, dtype=np.float32); b_c0 = np.asarray(b_c0, np.float32)
    W_fc = np.asarray(W_fc, dtype=np.float32); b_fc = np.asarray(b_fc, np.float32)

    with_bfc = bool(np.any(b_fc != 0))
    key = with_bfc
    if key not in _PROG_CACHE:
        _PROG_CACHE[key] = _build_program(with_bfc)
    nc = _PROG_CACHE[key]

    perm = _gate_perm()
    W_ih_p = W_ih[perm]
    W_hh_p = W_hh[perm]
    bsum_p = (b_ih + b_hh)[perm]

    wembT = _chunkT(W_ih_p[:, :E])
    wencT = _chunkT(W_ih_p[:, E:])
    whhT = _chunkT(W_hh_p)
    wh0T = _chunkT(W_h0 / np.float32(P))
    wc0T = _chunkT(W_c0 / np.float32(P))
    wfcT = _bf16(_chunkT(W_fc))
    bsum8 = np.tile(bsum_p, (R, 1)).astype(np.float32)
    bh032 = np.tile(b_h0, (32, 1)).astype(np.float32)
    bc032 = np.tile(b_c0, (32, 1)).astype(np.float32)
    rowsel = np.zeros((128, R * R), np.float32)
    for r in range(R):
        rowsel[:, 8 * r + r] = 1.0
    selm = np.zeros((128, 16 * 32), np.float32)
    for jj in range(16):
        selm[8 * jj:8 * jj + 8, 32 * jj:32 * (jj + 1)] = np.tile(
            np.eye(8, dtype=np.float32), (1, 4))
    irep8 = np.tile(np.eye(8, dtype=np.float32), (1, 4))

    in_maps = []
    all_rows = []
    for c in range(NCORES):
        rows = np.arange(c, B, NCORES)
        all_rows.append(rows)
        cap = np.asarray(encoder_captions[rows][:, :TD], dtype=np.int64)
        embg = embedding[cap]                       # [R, TD, E]
        embT = np.zeros((E, 512), np.float32)
        embT[:, :POS] = embg.transpose(2, 1, 0).reshape(E, POS)
        embT = np.ascontiguousarray(embT).reshape(KD, 128, 512)
        dec_len = (caption_len[rows] - 1).astype(np.int64)
        tt = np.arange(TD)[:, None]                 # [TD, 1]
        mpos = (tt < dec_len[None, :]).astype(np.float32).reshape(POS)
        maskB = np.ascontiguousarray(
            (tt < dec_len[None, :]).astype(np.float32).T)   # [R, TD]
        mask32 = np.tile(maskB, (4, 1))
        im = dict(enc=np.ascontiguousarray(encoder_out[rows]),
                  embT=embT, wembT=wembT, wencT=wencT, whhT=whhT,
                  wh0T=wh0T, wc0T=wc0T, wfcT=wfcT, bsum8=bsum8,
                  bh032=bh032, bc032=bc032, mask32=mask32, rowsel=rowsel,
                  selm=selm, irep8=irep8)
        if with_bfc:
            im["bfc"] = _bf16(b_fc.reshape(1, V))
            im["mask1"] = _bf16(mpos.reshape(1, POS))
        in_maps.append(im)

    global _LAST_IN_MAPS
    _LAST_IN_MAPS = in_maps
    res = run_bass_kernel_spmd(nc, in_maps, list(range(NCORES)))

    out = np.zeros((B, TD, V), np.float32)
    for c in range(NCORES):
        out[all_rows[c]] = res.results[c]["preds"]
    return out


# revision 27
# speedup vs baseline: 1.1344x; 1.1344x over previous
"""Trainium2 Bass kernel for an image-captioning LSTM decoder.

Model (per reference):
  emb = embedding[captions]                      [B, T, E]
  sum_enc = encoder_out.sum(axis=1)              [B, ENC]
  h0 = mean_enc @ W_h0.T + b_h0 ; c0 likewise
  per step t (Tdec = T-1 steps):
    gates = [emb_t, sum_enc] @ W_ih.T + b_ih + h @ W_hh.T + b_hh
    i,f,g,o -> LSTM update; rows with t >= caption_len-1 give preds 0
    preds_t = h_new @ W_fc.T + b_fc  (masked)

Sharding: data-parallel over batch: core c owns rows {c, c+8, ...}.

Per-core design (v2):
  - EG[t] = emb_t@W_ih[:, :E].T + sum_enc@W_ih[:, E:].T + biases precomputed
    for all steps as one GEMM (phase B), kept RESIDENT in SBUF (f32r),
    compact layout [128 positions(16 steps), 2048].
  - Gates PSUM is quarter-packed [128, 512]: gate q-map [g@0, o@32, i@64,
    f@96], each quarter holds 4 replicas of the 8 batch rows.  EG is folded
    into PSUM by a selector matmul (SEL lhsT, tile_position), then 4 k-chunk
    matmuls accumulate W_hh·h (HTrep stationary [128,32]).
  - Pointwise tail reads PSUM quarters directly (mixed-space ops dodge the
    equal-base rule): tanh g -> Bt[0:32]; sig [i|f] in one [64] ACT op;
    t1 = i*g (DVE), t2 = f*c (Pool) in parallel; c' = t1+t2; tanh; h =
    stt(tanh_c, mask, sig_o).  All ops are d-halved (256) so the next
    step's k0/k1 gate matmuls start as soon as HTrep k0/k1 are cast.
  - W_fc resident in SBUF as bf16 (80KB/partition); h additionally cast to
    bf16 (HTb) for the fc GEMM.  fc chunks (500 vocab cols) interleave into
    PE gaps; outputs stream to HBM per chunk.
"""

import numpy as np
from contextlib import ExitStack

import concourse.bass as bass
import concourse.tile as tile
from concourse import mybir, bacc
from concourse.bass_utils import run_bass_kernel_spmd
from concourse.masks import make_identity

F32 = mybir.dt.float32
F32R = mybir.dt.float32r
BF16 = mybir.dt.bfloat16
SIG = mybir.ActivationFunctionType.Sigmoid
TANH = mybir.ActivationFunctionType.Tanh

NCORES = 8
B, T, V, E, D, ENC, P = 64, 64, 10000, 512, 512, 512, 196
TD = T - 1            # 63 decode steps
R = B // NCORES       # 8 rows per core
POS = TD * R          # 504 positions per core
KD = D // 128         # 4 k-tiles of the hidden dim
VCH = 500             # vocab chunk (psum bank holds 512 fp32)
NV = V // VCH         # 20 chunks
MTS = [128, 128, 128, POS - 384]   # fc position tiles (504 = 3*128 + 120)

_PROG_CACHE = {}


def _build_program(with_bfc: bool):
    nc = bacc.Bacc("TRN2", target_bir_lowering=False, debug=False,
                   num_devices=NCORES)

    def inp(name, shape, dt=F32):
        return nc.dram_tensor(name, shape, dt, kind="ExternalInput").ap()

    enc = inp("enc", [R, P, ENC], F32R)
    embT = inp("embT", [KD, 128, 512], F32R)
    wembT = inp("wembT", [KD, 128, 4 * D], F32R)
    wencT = inp("wencT", [KD, 128, 4 * D], F32R)
    whhT = inp("whhT", [KD, 128, 4 * D], F32R)
    wh0T = inp("wh0T", [KD, 128, D], F32R)
    wc0T = inp("wc0T", [KD, 128, D], F32R)
    wfcT = inp("wfcT", [KD, 128, V], BF16)
    bsum8 = inp("bsum8", [R, 4 * D])
    bh032 = inp("bh032", [32, D])
    bc032 = inp("bc032", [32, D])
    mask32 = inp("mask32", [32, TD])
    rowsel = inp("rowsel", [128, R * R], F32R)
    selm = inp("selm", [128, 16 * 32], F32R)
    irep8 = inp("irep8", [8, 32], F32R)
    if with_bfc:
        bfc = inp("bfc", [1, V], BF16)
        mask1 = inp("mask1", [1, POS], BF16)
    preds = nc.dram_tensor("preds", [R, TD, V], F32, kind="ExternalOutput").ap()
    preds_trv = preds.rearrange("r t v -> t r v")

    with tile.TileContext(nc) as tc, ExitStack() as ctx:
        const_pool = ctx.enter_context(tc.tile_pool(name="const", bufs=1))
        state_pool = ctx.enter_context(tc.tile_pool(name="state", bufs=1))

        SEL = const_pool.tile([128, 16 * 32], F32R, name="SEL")
        nc.sync.dma_start(SEL[:], selm[:])
        IREP = const_pool.tile([8, 32], F32R, name="IREP")
        nc.sync.dma_start(IREP[:], irep8[:])
        maskb = const_pool.tile([32, TD], F32, name="maskb")
        nc.sync.dma_start(maskb[:], mask32[:])

        ENGS = [nc.sync, nc.gpsimd, nc.scalar, nc.sync]
        whh_sb = [const_pool.tile([128, 4 * D], F32R, name=f"whh{k}")
                  for k in range(KD)]
        for k in range(KD):
            ENGS[k].dma_start(whh_sb[k][:], whhT[k])

        # persistent state
        EG = [state_pool.tile([128, 4 * D], F32R, name=f"EG{m}")
              for m in range(4)]

        HTb = [state_pool.tile([128, R * T], BF16, name=f"HTb{k}")
               for k in range(KD)]
        Gt = state_pool.tile([32, D], F32, name="Gt")   # tanh(g)
        Ct = state_pool.tile([32, D], F32, name="Ct")   # c state
        xenc16 = state_pool.tile([128, 4 * D], F32, name="xenc16")
        h0_sb = state_pool.tile([32, D], F32R, name="h0_sb")
        HTrep = [state_pool.tile([128, 128], F32R, name=f"HTrep{s}")
                 for s in range(2)]

        # ---- Phase A: encoder row sums, h0/c0, xenc ----
        with tc.tile_pool(name="pha", bufs=1) as pha, \
             tc.tile_pool(name="phas", bufs=2) as phas, \
             tc.tile_pool(name="pha_ps", bufs=1, space="PSUM") as pha_ps, \
             tc.tile_pool(name="phb_ps", bufs=1, space="PSUM") as phb_ps:
            rsel = pha.tile([128, R * R], F32R, name="rsel")
            nc.sync.dma_start(rsel[:], rowsel[:])

            sum_ps = pha_ps.tile([R, ENC], F32, name="sum_ps")
            nmm = 0
            for r in range(R):
                for c2, (p0, pn) in enumerate(((0, 128), (128, P - 128))):
                    et = phas.tile([128, ENC], F32R, name=f"enc{r}_{c2}",
                                  tag="encchunk")
                    ENGS[r % 4].dma_start(et[:pn, :], enc[r, p0:p0 + pn, :])
                    nc.tensor.matmul(sum_ps[:], rsel[:pn, 8 * r:8 * r + R],
                                     et[:pn, :], start=(nmm == 0),
                                     stop=(nmm == 2 * R - 1))
                    nmm += 1
            sum_sb = pha.tile([R, ENC], F32R, name="sum_sb")
            nc.vector.tensor_copy(sum_sb[:], sum_ps[:])

            # sumTrep[k]: [128, 32] transposed sums, 4 replicas
            sumTrep = []
            for k in range(KD):
                tp = phb_ps.tile([128, 32], F32, name=f"sumT_ps{k}", tag="tps")
                nc.tensor.matmul(tp[:], sum_sb[:, 128 * k:128 * (k + 1)],
                                 IREP[:], start=True, stop=True)
                st = pha.tile([128, 32], F32R, name=f"sumTrep{k}")
                nc.vector.tensor_copy(st[:], tp[:])
                sumTrep.append(st)

            # h0 / c0 (weights pre-scaled by 1/196 on host), [32, D] replicated
            for name, wT, b32 in (("h0", wh0T, bh032), ("c0", wc0T, bc032)):
                ps = phb_ps.tile([32, D], F32, name=f"{name}_ps", tag="h0ps")
                for k in range(KD):
                    wt = phas.tile([128, D], F32R, name=f"w{name}{k}",
                                   tag="w0chunk")
                    ENGS[k].dma_start(wt[:], wT[k])
                    nc.tensor.matmul(ps[:], sumTrep[k][:], wt[:],
                                     start=(k == 0), stop=(k == KD - 1))
                bt = pha.tile([32, D], F32, name=f"b{name}", tag="b0chunk")
                nc.sync.dma_start(bt[:], b32[:])
                if name == "h0":
                    nc.vector.tensor_add(h0_sb[:], ps[:], bt[:])
                else:
                    nc.vector.tensor_add(Ct[:], ps[:], bt[:])

            # xenc = sum_enc @ W_ih[:, E:].T + bsum  -> [8, 2048]
            xe_ps = pha_ps.tile([R, 4 * D], F32, name="xe_ps", tag="bigps")
            for k in range(KD):
                wt = phas.tile([128, 4 * D], F32R, name=f"wenc{k}",
                               tag="wencchunk")
                ENGS[k].dma_start(wt[:], wencT[k])
                for n in range(4):
                    nc.tensor.matmul(xe_ps[:, D * n:D * (n + 1)],
                                     sumTrep[k][:, 0:8],
                                     wt[:, D * n:D * (n + 1)],
                                     start=(k == 0), stop=(k == KD - 1))
            bs = pha.tile([R, 4 * D], F32, name="bsum_sb")
            nc.sync.dma_start(bs[:], bsum8[:])
            xenc = pha.tile([R, 4 * D], F32, name="xenc")
            nc.vector.tensor_add(xenc[:], xe_ps[:], bs[:])
            nc.sync.dma_start(xenc16[0:8, :], xenc[:])
            for w in (8, 16, 32, 64):
                nc.sync.dma_start(xenc16[w:2 * w, :], xenc16[0:w, :])

            # h0 -> HTrep[0] via replicated transpose (do inside step pools)

        # ---- Phase B: EG = embT.T @ wembT (+xenc16) -> SBUF resident ----
        with tc.tile_pool(name="phb", bufs=1) as phb, \
             tc.tile_pool(name="phb2_ps", bufs=2, space="PSUM") as phb2_ps:
            emb_sb = [phb.tile([128, 512], F32R, name=f"emb_sb{k}")
                      for k in range(KD)]
            wemb_sb = [phb.tile([128, 4 * D], F32R, name=f"wemb_sb{k}")
                       for k in range(KD)]
            for k in range(KD):
                ENGS[k].dma_start(emb_sb[k][:], embT[k])
                ENGS[(k + 2) % 4].dma_start(wemb_sb[k][:], wembT[k])
            for m in range(4):
                eg_ps = phb2_ps.tile([128, 4 * D], F32, name=f"eg_ps{m}",
                                     tag="egps")
                for n in range(4):
                    for k in range(KD):
                        nc.tensor.matmul(
                            eg_ps[:, D * n:D * (n + 1)],
                            emb_sb[k][:, 128 * m:128 * (m + 1)],
                            wemb_sb[k][:, D * n:D * (n + 1)],
                            start=(k == 0), stop=(k == KD - 1))
                nc.vector.tensor_add(EG[m][:, :], eg_ps[:, :],
                                     xenc16[:, :])

        # ---- Steps ----
        with tc.tile_pool(name="wfc", bufs=1) as wfc_pool, \
             tc.tile_pool(name="phc", bufs=2) as phc, \
             tc.tile_pool(name="phd_out", bufs=3) as phd_out, \
             tc.tile_pool(name="gps", bufs=1, space="PSUM") as gps_pool, \
             tc.tile_pool(name="tps", bufs=1, space="PSUM") as tps_pool, \
             tc.tile_pool(name="flps", bufs=1, space="PSUM") as fl_pool, \
             tc.tile_pool(name="fps", bufs=2, space="PSUM") as fps_pool:

            wfc_sb = [wfc_pool.tile([128, V], BF16, name=f"wfc{k}")
                      for k in range(KD)]
            ENGS2 = [nc.sync, nc.gpsimd, nc.scalar, nc.gpsimd]
            for k in range(KD):
                ENGS2[k].dma_start(wfc_sb[k][:], wfcT[k])
            if with_bfc:
                bfc_sb = wfc_pool.tile([1, V], BF16, name="bfc_sb")
                nc.sync.dma_start(bfc_sb[:], bfc[:])
                m1_sb = wfc_pool.tile([1, POS], BF16, name="m1_sb")
                nc.sync.dma_start(m1_sb[:], mask1[:])

            # h0 -> HTrep[0] via replicated transposes
            init_tp = tps_pool.tile([128, 128], F32, name="init_tp", tag="tp")
            for k in range(KD):
                nc.tensor.matmul(init_tp[:, 32 * k:32 * (k + 1)],
                                 h0_sb[0:8, 128 * k:128 * (k + 1)], IREP[:],
                                 start=True, stop=True)
            nc.vector.tensor_copy(HTrep[0][:], init_tp[:])

            def fc_mm(m, n):
                """fc GEMM for preds[postile m, vocab chunk n] -> psum."""
                mw = MTS[m]
                ps = fps_pool.tile([128, VCH], F32, name=f"fc{n}_{m}",
                                   tag="fcps")
                for k in range(KD):
                    nc.tensor.matmul(
                        ps[:mw, :],
                        HTb[k][:, R + 128 * m:R + 128 * m + mw],
                        wfc_sb[k][:, VCH * n:VCH * (n + 1)], start=(k == 0),
                        stop=(k == KD - 1 and not with_bfc))
                if with_bfc:
                    nc.tensor.matmul(
                        ps[:mw, :], m1_sb[:, 128 * m:128 * m + mw],
                        bfc_sb[:, VCH * n:VCH * (n + 1)],
                        start=False, stop=True)
                return ps

            def fc_out(ps, m, n):
                mw = MTS[m]
                ot = phd_out.tile([128, VCH], F32, name=f"fo{n}_{m}",
                                  tag="fcout")
                nc.vector.tensor_copy(ot[:mw, :], ps[:mw, :])
                t0 = 16 * m
                tn = mw // R
                nc.sync.dma_start(
                    preds_trv[t0:t0 + tn, :, VCH * n:VCH * (n + 1)],
                    ot[:mw, :])

            def emit_fc_mm(t):
                out = []
                m = t // 16 - 1
                if m >= 0:
                    L = min(16 * (m + 2), TD) - 16 * (m + 1)
                    s = t - 16 * (m + 1)
                    for n in range(s * NV // L, (s + 1) * NV // L):
                        out.append((fc_mm(m, n), m, n))
                return out

            GORD = (0, 3, 2, 1)  # emission order: g, f, i, o
            NFILL = 6

            gps_tiles = {}

            def bankof(t):
                return [gps_pool.tile([32, D], F32, name=f"ps{t}_{g}",
                                      tag=f"gps{g}")
                        for g in range(4)]

            banks = {}
            fillps = fl_pool.tile([128, D], F32, name="fillps", tag="fill")

            def fill(n):
                for _ in range(n):
                    nc.tensor.matmul(fillps[:], whh_sb[0][:, 0:128],
                                     whh_sb[1][:, 0:D], start=True,
                                     stop=True)

            hprev = [None]
            for t in range(TD):
                htr = HTrep[t % 2]
                j = t % 16
                mt = t // 16
                banks[t] = bankof(t)
                bank = banks[t]
                # --- PE: transposes(t-1) + cast, fill, fc, gate regions ---
                if t > 0:
                    hp, tpp = hprev[0]
                    for k in range(KD):
                        nc.tensor.matmul(tpp[:, 32 * k:32 * (k + 1)],
                                         hp[0:8, 128 * k:128 * (k + 1)],
                                         IREP[:], start=True, stop=True)
                    nc.vector.tensor_copy(htr[:], tpp[:])
                    for k in range(KD):
                        nc.gpsimd.tensor_copy(
                            HTb[k][:, R * t:R * (t + 1)],
                            htr[:, 32 * k:32 * k + 8])
                    if NFILL:
                        fill(NFILL)
                fcq = emit_fc_mm(t)
                for g in GORD:
                    gsl = slice(512 * g, 512 * (g + 1))
                    nc.tensor.matmul(bank[g][:], SEL[:, 32 * j:32 * (j + 1)],
                                     EG[mt][:, gsl], start=True, stop=False)
                    for k in range(KD):
                        nc.tensor.matmul(bank[g][:],
                                         htr[:, 32 * k:32 * (k + 1)],
                                         whh_sb[k][:, gsl],
                                         start=False, stop=(k == KD - 1))

                # --- tail ---
                a_i = phc.tile([32, D], F32, name=f"ai{t}", tag="ai")
                a_f = phc.tile([32, D], F32, name=f"af{t}", tag="af")
                acto = phc.tile([32, D], F32, name=f"ao{t}", tag="ao")
                t1s = phc.tile([32, D], F32, name=f"t1_{t}", tag="t1s")
                t2s = phc.tile([32, D], F32, name=f"t2_{t}", tag="t2s")
                tc_t = phc.tile([32, D], F32, name=f"tc{t}", tag="tct")
                h_sb = phc.tile([32, D], F32R, name=f"h{t}", tag="hsb")
                tp = tps_pool.tile([128, 128], F32, name=f"tp{t}", tag="tp")
                nc.scalar.activation(Gt[:], bank[0][:], TANH)
                nc.scalar.activation(a_f[:], bank[3][:], SIG)
                nc.scalar.activation(a_i[:], bank[2][:], SIG)
                nc.scalar.activation(acto[:], bank[1][:], SIG)
                nc.gpsimd.tensor_mul(t2s[:], a_f[:], Ct[:])
                nc.vector.tensor_mul(t1s[:], a_i[:], Gt[:])
                nc.vector.tensor_add(Ct[:], t1s[:], t2s[:])
                nc.scalar.activation(tc_t[:], Ct[:], TANH)
                nc.vector.scalar_tensor_tensor(
                    h_sb[:], tc_t[:], maskb[:, t:t + 1], acto[:],
                    mybir.AluOpType.mult, mybir.AluOpType.mult)
                hprev[0] = (h_sb, tp)
                for args in fcq:
                    fc_out(*args)

            # last step's h -> HTb col TD (for the final fc m=3 chunks)
            hp, tpp = hprev[0]
            for k in range(KD):
                nc.tensor.matmul(tpp[:, 32 * k:32 * (k + 1)],
                                 hp[0:8, 128 * k:128 * (k + 1)],
                                 IREP[:], start=True, stop=True)
            nc.vector.tensor_copy(HTrep[TD % 2][:], tpp[:])
            for k in range(KD):
                nc.gpsimd.tensor_copy(HTb[k][:, R * TD:R * (TD + 1)],
                                      HTrep[TD % 2][:, 32 * k:32 * k + 8])

            for n in range(NV):
                fc_out(fc_mm(3, n), 3, n)

    nc.compile()
    return nc


GPERM = None  # row permutation [g, o, i, f] built lazily


def _gate_perm():
    global GPERM
    if GPERM is None:
        GPERM = np.concatenate([np.arange(2 * D, 3 * D),
                                np.arange(3 * D, 4 * D),
                                np.arange(0, D), np.arange(D, 2 * D)])
    return GPERM


def _chunkT(w):
    """[N, K<=512] weight -> transposed chunks [KD, 128, N] (contiguous)."""
    wt = np.ascontiguousarray(w.T.astype(np.float32))
    return wt.reshape(KD, 128, w.shape[0])


def _bf16(a):
    import ml_dtypes
    return a.astype(ml_dtypes.bfloat16)


def kernel(encoder_out, encoder_captions, caption_len, embedding,
           W_ih, b_ih, W_hh, b_hh, W_h0, b_h0, W_c0, b_c0, W_fc, b_fc):
    encoder_out = np.asarray(encoder_out, dtype=np.float32)
    encoder_captions = np.asarray(encoder_captions)
    caption_len = np.asarray(caption_len)
    embedding = np.asarray(embedding, dtype=np.float32)
    W_ih = np.asarray(W_ih, dtype=np.float32); b_ih = np.asarray(b_ih, np.float32)
    W_hh = np.asarray(W_hh, dtype=np.float32); b_hh = np.asarray(b_hh, np.float32)
    W_h0 = np.asarray(W_h0, dtype=np.float32); b_h0 = np.asarray(b_h0, np.float32)
    W_c0 = np.asarray(W_c0, dtype=np.float32); b_c0 = np.asarray(b_c0, np.float32)
    W_fc = np.asarray(W_fc, dtype=np.float32); b_fc = np.asarray(b_fc, np.float32)

    with_bfc = bool(np.any(b_fc != 0))
    key = with_bfc
    if key not in _PROG_CACHE:
        _PROG_CACHE[key] = _build_program(with_bfc)
    nc = _PROG_CACHE[key]

    perm = _gate_perm()
    W_ih_p = W_ih[perm]
    W_hh_p = W_hh[perm]
    bsum_p = (b_ih + b_hh)[perm]

    wembT = _chunkT(W_ih_p[:, :E])
    wencT = _chunkT(W_ih_p[:, E:])
    whhT = _chunkT(W_hh_p)
    wh0T = _chunkT(W_h0 / np.float32(P))
    wc0T = _chunkT(W_c0 / np.float32(P))
    wfcT = _bf16(_chunkT(W_fc))
    bsum8 = np.tile(bsum_p, (R, 1)).astype(np.float32)
    bh032 = np.tile(b_h0, (32, 1)).astype(np.float32)
    bc032 = np.tile(b_c0, (32, 1)).astype(np.float32)
    rowsel = np.zeros((128, R * R), np.float32)
    for r in range(R):
        rowsel[:, 8 * r + r] = 1.0
    selm = np.zeros((128, 16 * 32), np.float32)
    for jj in range(16):
        selm[8 * jj:8 * jj + 8, 32 * jj:32 * (jj + 1)] = np.tile(
            np.eye(8, dtype=np.float32), (1, 4))
    irep8 = np.tile(np.eye(8, dtype=np.float32), (1, 4))

    in_maps = []
    all_rows = []
    for c in range(NCORES):
        rows = np.arange(c, B, NCORES)
        all_rows.append(rows)
        cap = np.asarray(encoder_captions[rows][:, :TD], dtype=np.int64)
        embg = embedding[cap]                       # [R, TD, E]
        embT = np.zeros((E, 512), np.float32)
        embT[:, :POS] = embg.transpose(2, 1, 0).reshape(E, POS)
        embT = np.ascontiguousarray(embT).reshape(KD, 128, 512)
        dec_len = (caption_len[rows] - 1).astype(np.int64)
        tt = np.arange(TD)[:, None]                 # [TD, 1]
        mpos = (tt < dec_len[None, :]).astype(np.float32).reshape(POS)
        maskB = np.ascontiguousarray(
            (tt < dec_len[None, :]).astype(np.float32).T)   # [R, TD]
        mask32 = np.tile(maskB, (4, 1))
        im = dict(enc=np.ascontiguousarray(encoder_out[rows]),
                  embT=embT, wembT=wembT, wencT=wencT, whhT=whhT,
                  wh0T=wh0T, wc0T=wc0T, wfcT=wfcT, bsum8=bsum8,
                  bh032=bh032, bc032=bc032, mask32=mask32, rowsel=rowsel,
                  selm=selm, irep8=irep8)
        if with_bfc:
            im["bfc"] = _bf16(b_fc.reshape(1, V))
            im["mask1"] = _bf16(mpos.reshape(1, POS))
        in_maps.append(im)

    global _LAST_IN_MAPS
    _LAST_IN_MAPS = in_maps
    res = run_bass_kernel_spmd(nc, in_maps, list(range(NCORES)))

    out = np.zeros((B, TD, V), np.float32)
    for c in range(NCORES):
        out[all_rows[c]] = res.results[c]["preds"]
    return out
